# revision 6
# baseline (speedup 1.0000x reference)
"""Chamfer distance kernel for Trainium2, 8 NeuronCores.

Problem: points1 (16384,3) f32, points2 (16384,3) f32.
  dist = cdist(points1, points2); out = sum(min(dist,axis=1)) + sum(min(dist,axis=0))

Strategy:
  - Shard points1 rows across the 8 cores (2048 rows each); points2 replicated.
  - d2[i,j] = ||p1_i||^2 + ||p2_j||^2 - 2 p1_i.p2_j is produced directly by a
    K=5 augmented fp32 matmul on TensorE:
        lhsT rows = [x1, y1, z1, 1, sq1],  rhs rows = [-2x2, -2y2, -2z2, sq2, 1]
  - ScalarE casts each PSUM d2 block to fp16 in SBUF; VectorE accumulates
    row minima and column minima with fp16 tensor_tensor(min) at 2x rate.
    (min commutes with the monotone sqrt, so sqrt is applied at the end to
    the 16384+16384 minima only, on host, in f64.)
  - Column minima need a cross-partition reduce: PE-transpose (fp16) of the
    column accumulator + free-axis reduce.
  - Per-core partial outputs (row minima of its row block, column-minima
    partial over its rows) are combined on host: min across cores, sqrt, sum.
"""

import sys
from contextlib import ExitStack

import numpy as np

sys.path.insert(0, "/opt/trn_rl_repo")

import concourse.bass as bass  # noqa: E402
import concourse.tile as tile  # noqa: E402
from concourse import mybir  # noqa: E402
from concourse.masks import make_identity  # noqa: E402

F32 = mybir.dt.float32
F16 = mybir.dt.float16
MIN = mybir.AluOpType.min


def _split_multi_waits(bir: dict, max_waits: int = 1) -> dict:
    """Hoist extra sync waits into standalone EventSemaphore instructions.

    The walrus in this container encodes at most one sync-wait command per
    instruction (raw bass emits standalone EventSemaphore waits for the same
    reason); Tile attaches every required wait to the consuming instruction.
    Splitting preserves semantics exactly: the engine stalls at the hoisted
    wait(s), then executes the op with the remaining wait.
    """
    for fn in bir["functions"]:
        for blk in fn["blocks"]:
            new = []
            for inst in blk["instructions"]:
                si = inst.get("sync_info")
                waits = (si or {}).get("on_wait") or []
                if si is not None and len(waits) > max_waits:
                    for k, w in enumerate(waits[:-max_waits]):
                        new.append({
                            "debug": inst.get("debug", 0),
                            "engine": inst["engine"],
                            "ins": [], "outs": [],
                            "name": f'{inst["name"]}-hw{k}',
                            "opcode": "EventSemaphore",
                            "sync_info": {"on_update": [], "on_wait": [w]},
                        })
                    si["on_wait"] = waits[-max_waits:]
                new.append(inst)
            blk["instructions"] = new
    return bir


def _install_wait_splitter(nc: bass.Bass) -> None:
    import orjson

    orig = nc.to_json_bytes

    def patched() -> bytes:
        return orjson.dumps(_split_multi_waits(orjson.loads(orig())))

    nc.to_json_bytes = patched

N_CORES = 8
N1_FULL = 16384
N2_FULL = 16384
P = 128  # partitions
MM_N = 512  # matmul free dim (one PSUM bank of f32)


def build_chamfer_bass(n1: int, n2: int, gj: int = 2048) -> bass.Bass:
    """Build the per-core Bass program.

    Per-core inputs:
      p1aug [5, n1]  f32 : rows [x1, y1, z1, 1, sq1] for this core's rows
      p2aug [5, n2]  f32 : rows [-2x2, -2y2, -2z2, sq2, 1] (replicated)
    Per-core outputs:
      rowmin [128, n1//128] f32 : rowmin[p, it] = min_j d2[it*128+p, j]
      colmin [128, n2//128] f32 : colmin[p, c]  = min_{i in core rows} d2[i, c*128+p]
    """
    assert n1 % P == 0 and n2 % gj == 0 and gj % MM_N == 0
    n_it = n1 // P
    n_jg = n2 // gj
    nb = gj // MM_N
    n_tp = gj // P  # transposes per j-group

    nc = bass.Bass("TRN2", target_bir_lowering=False, debug=False,
                   num_devices=N_CORES)

    p1aug = nc.dram_tensor("p1aug", [5, n1], F32, kind="ExternalInput").ap()
    p2aug = nc.dram_tensor("p2aug", [5, n2], F32, kind="ExternalInput").ap()
    rowmin_d = nc.dram_tensor("rowmin", [P, n_it], F32, kind="ExternalOutput").ap()
    colmin_d = nc.dram_tensor("colmin", [P, n2 // P], F32, kind="ExternalOutput").ap()

    with tile.TileContext(nc) as tc, ExitStack() as ctx:
        singles = ctx.enter_context(tc.tile_pool(name="singles", bufs=1))
        psum_pool = ctx.enter_context(tc.tile_pool(name="psum", bufs=2, space="PSUM"))
        d2c_pool = ctx.enter_context(tc.tile_pool(name="d2c", bufs=3))
        acc_pool = ctx.enter_context(tc.tile_pool(name="acc", bufs=2))

        lhs_sb = singles.tile([5, n1], F32)
        rhs_sb = singles.tile([5, n2], F32)
        # SWDGE (single queue) so the first matmul's wait fits the ISA's
        # sync-wait slots; HWDGE fans out to several queue semaphores.
        nc.gpsimd.dma_start(out=lhs_sb, in_=p1aug)
        nc.gpsimd.dma_start(out=rhs_sb, in_=p2aug)

        identity = singles.tile([P, P], F16)
        make_identity(nc, identity)

        colacc = singles.tile([P, n2], F16)
        rowmin_sb = singles.tile([P, n_it], F32)
        colmin_sb = singles.tile([P, n2 // P], F32)

        for it in range(n_it):
            rowacc = acc_pool.tile([P, gj], F16, tag="rowacc")
            lhs_slice = lhs_sb[:, it * P:(it + 1) * P]
            for jg in range(n_jg):
                ps = psum_pool.tile([P, gj], F32, tag="mm")
                for g in range(nb):
                    nc.tensor.matmul(
                        ps[:, g * MM_N:(g + 1) * MM_N],
                        lhs_slice,
                        rhs_sb[:, jg * gj + g * MM_N: jg * gj + (g + 1) * MM_N],
                        start=True, stop=True,
                    )
                d2c = d2c_pool.tile([P, gj], F16, tag="d2c")
                nc.scalar.copy(out=d2c, in_=ps)

                if jg == 0:
                    nc.vector.tensor_copy(out=rowacc, in_=d2c)
                else:
                    nc.vector.tensor_tensor(out=rowacc, in0=rowacc, in1=d2c, op=MIN)

                cslice = colacc[:, jg * gj:(jg + 1) * gj]
                if it == 0:
                    nc.vector.tensor_copy(out=cslice, in_=d2c)
                else:
                    nc.vector.tensor_tensor(out=cslice, in0=cslice, in1=d2c, op=MIN)

            nc.vector.tensor_reduce(
                out=rowmin_sb[:, it:it + 1], in_=rowacc,
                axis=mybir.AxisListType.X, op=MIN,
            )

        # Column minima: cross-partition reduce via PE transpose.
        for jg in range(n_jg):
            pst = psum_pool.tile([P, n_tp, P], F16, tag="mm")
            for t in range(n_tp):
                nc.tensor.transpose(
                    pst[:, t],
                    colacc[:, jg * gj + t * P: jg * gj + (t + 1) * P],
                    identity,
                )
            nc.vector.tensor_reduce(
                out=colmin_sb[:, jg * n_tp:(jg + 1) * n_tp], in_=pst,
                axis=mybir.AxisListType.X, op=MIN,
            )

        nc.sync.dma_start(out=rowmin_d, in_=rowmin_sb)
        nc.sync.dma_start(out=colmin_d, in_=colmin_sb)

    _install_wait_splitter(nc)
    return nc


def make_aug_inputs(points1: np.ndarray, points2: np.ndarray):
    """Host-side layout prep: augmented transposed operands per core."""
    p1 = np.ascontiguousarray(points1, dtype=np.float32)
    p2 = np.ascontiguousarray(points2, dtype=np.float32)
    n1, n2 = p1.shape[0], p2.shape[0]
    sq1 = (p1 * p1).sum(axis=1)
    sq2 = (p2 * p2).sum(axis=1)
    ones2 = np.ones(n2, dtype=np.float32)
    p2aug = np.ascontiguousarray(
        np.stack([-2.0 * p2[:, 0], -2.0 * p2[:, 1], -2.0 * p2[:, 2], sq2, ones2])
    )
    shard = n1 // N_CORES
    in_maps = []
    for c in range(N_CORES):
        s = slice(c * shard, (c + 1) * shard)
        p1c, sq1c = p1[s], sq1[s]
        ones1 = np.ones(shard, dtype=np.float32)
        p1aug = np.ascontiguousarray(
            np.stack([p1c[:, 0], p1c[:, 1], p1c[:, 2], ones1, sq1c])
        )
        in_maps.append({"p1aug": p1aug, "p2aug": p2aug})
    return in_maps


def combine_outputs(results: list) -> np.ndarray:
    """Host-side unshard: gather per-core minima, final min/sqrt/sum."""
    rowmins = np.concatenate(
        [np.asarray(r["rowmin"]).T.reshape(-1) for r in results]
    )  # (n1,) squared distances
    colmin = np.stack(
        [np.asarray(r["colmin"]).T.reshape(-1) for r in results]
    ).min(axis=0)  # (n2,)
    total = (np.sqrt(np.maximum(rowmins, 0.0, dtype=np.float64)).sum()
             + np.sqrt(np.maximum(colmin, 0.0, dtype=np.float64)).sum())
    return np.float32(total)


_CACHED = {}


def kernel(points1: np.ndarray, points2: np.ndarray) -> np.ndarray:
    from concourse.bass_utils import run_bass_kernel_spmd

    key = (points1.shape, points2.shape)
    if key not in _CACHED:
        _CACHED[key] = build_chamfer_bass(points1.shape[0] // N_CORES,
                                          points2.shape[0])
    nc = _CACHED[key]
    in_maps = make_aug_inputs(np.asarray(points1), np.asarray(points2))
    res = run_bass_kernel_spmd(nc, in_maps, list(range(N_CORES)))
    return combine_outputs(res.results)


def _numpy_ref(p1: np.ndarray, p2: np.ndarray, chunk: int = 1024) -> float:
    """Chunked numpy chamfer reference (f32 matmul expansion like the oracle)."""
    sq1 = (p1 * p1).sum(1)
    sq2 = (p2 * p2).sum(1)
    rowmin = np.full(p1.shape[0], np.inf, np.float32)
    colmin = np.full(p2.shape[0], np.inf, np.float32)
    for s in range(0, p1.shape[0], chunk):
        d2 = (sq1[s:s + chunk, None] + sq2[None, :]
              - 2.0 * (p1[s:s + chunk] @ p2.T))
        rowmin[s:s + chunk] = d2.min(1)
        np.minimum(colmin, d2.min(0), out=colmin)
    return float(np.sqrt(np.maximum(rowmin, 0)).sum()
                 + np.sqrt(np.maximum(colmin, 0)).sum())


if __name__ == "__main__":
    rng = np.random.default_rng(0)
    p1 = rng.standard_normal((N1_FULL, 3), dtype=np.float32)
    p2 = rng.standard_normal((N2_FULL, 3), dtype=np.float32)
    out = kernel(p1, p2)
    ref = _numpy_ref(p1, p2)
    print("kernel:", out, "ref:", ref, "rel:", abs(out - ref) / abs(ref))


# revision 21
# speedup vs baseline: 1.0710x; 1.0710x over previous
"""Chamfer distance kernel for Trainium2, 8 NeuronCores.

Problem: points1 (16384,3) f32, points2 (16384,3) f32.
  dist = cdist(points1, points2); out = sum(min(dist,axis=1)) + sum(min(dist,axis=0))

Strategy:
  - Shard points1 rows across the 8 cores (2048 rows each); points2 replicated.
  - d2[i,j] = ||p1_i||^2 + ||p2_j||^2 - 2 p1_i.p2_j is produced directly by an
    augmented matmul on TensorE with A = [x1, y1, z1, 1, sq1] against
    B = [-2x2, -2y2, -2z2, sq2, 1]. fp32 matmuls stream at 1/4 PE rate, so
    A and B are hi/lo-split into fp16 and fed as one K=20 fp16 matmul
    (hiA+loA).(hiB+loB) — exact products, f32 PSUM accumulation, full rate.
  - ScalarE casts each PSUM d2 block to fp16 in SBUF; VectorE accumulates
    row minima and column minima with fp16 tensor_tensor(min) at 2x rate.
    (min commutes with the monotone sqrt, so sqrt is applied at the end to
    the 16384+16384 minima only, on host, in f64.)
  - Column minima need a cross-partition reduce: PE-transpose (fp16) of the
    column accumulator + free-axis reduce.
  - Per-core partial outputs (row minima of its row block, column-minima
    partial over its rows) are combined on host: min across cores, sqrt, sum.
"""

import sys
from contextlib import ExitStack

import numpy as np

sys.path.insert(0, "/opt/trn_rl_repo")

import concourse.bass as bass  # noqa: E402
import concourse.tile as tile  # noqa: E402
from concourse import mybir  # noqa: E402
from concourse.masks import make_identity  # noqa: E402

F32 = mybir.dt.float32
F16 = mybir.dt.float16
MIN = mybir.AluOpType.min


def _split_multi_waits(bir: dict, max_waits: int = 1) -> dict:
    """Hoist extra sync waits into standalone EventSemaphore instructions.

    The walrus in this container encodes at most one sync-wait command per
    instruction (raw bass emits standalone EventSemaphore waits for the same
    reason); Tile attaches every required wait to the consuming instruction.
    Splitting preserves semantics exactly: the engine stalls at the hoisted
    wait(s), then executes the op with the remaining wait.
    """
    for fn in bir["functions"]:
        for blk in fn["blocks"]:
            new = []
            for inst in blk["instructions"]:
                si = inst.get("sync_info")
                waits = (si or {}).get("on_wait") or []
                if si is not None and len(waits) > max_waits:
                    for k, w in enumerate(waits[:-max_waits]):
                        new.append({
                            "debug": inst.get("debug", 0),
                            "engine": inst["engine"],
                            "ins": [], "outs": [],
                            "name": f'{inst["name"]}-hw{k}',
                            "opcode": "EventSemaphore",
                            "sync_info": {"on_update": [], "on_wait": [w]},
                        })
                    si["on_wait"] = waits[-max_waits:]
                new.append(inst)
            blk["instructions"] = new
    return bir


def _install_wait_splitter(nc: bass.Bass) -> None:
    import orjson

    orig = nc.to_json_bytes

    def patched() -> bytes:
        return orjson.dumps(_split_multi_waits(orjson.loads(orig())))

    nc.to_json_bytes = patched

N_CORES = 8
N1_FULL = 16384
N2_FULL = 16384
P = 128  # partitions
MM_N = 512  # matmul free dim (one PSUM bank of f32)


def build_chamfer_bass(n1: int, n2: int, gj: int = 2048, repeat: int = 1,
                       use_ttr: bool = False, d2c_bufs: int = 3,
                       psum_bufs: int = 2) -> bass.Bass:
    # use_ttr=True (fused TensorTensorReduce rowmin) validates in CoreSim but
    # this container's walrus rejects the custom DVE ISA op ("ISA wrong
    # length"), so it stays off.
    """Build the per-core Bass program.

    fp32 matmuls stream at 1/4 rate on the PE, so the K=5 augmented operands
    are split hi/lo into fp16 and fed as a K=20 fp16 matmul computing
    (hiA+loA).(hiB+loB) exactly (fp16 products are exact in the f32
    accumulator), i.e. fp32-level d2 at full PE streaming rate.

    Per-core inputs:
      p1aug [20, n1] f16 : rows [hiA, hiA, loA, loA], A = [x1, y1, z1, 1, sq1]
      p2aug [20, n2] f16 : rows [hiB, loB, hiB, loB], B = [-2x2, -2y2, -2z2, sq2, 1]
    Per-core outputs:
      rowmin [128, n1//128] f32 : rowmin[p, it] = min_j d2[it*128+p, j]
      colmin [128, n2//128] f32 : colmin[p, c]  = min_{i in core rows} d2[i, c*128+p]
    """
    assert n1 % P == 0 and n2 % gj == 0 and gj % MM_N == 0
    n_it = n1 // P
    n_jg = n2 // gj
    nb = gj // MM_N
    n_tp = gj // P  # transposes per j-group

    nc = bass.Bass("TRN2", target_bir_lowering=False, debug=False,
                   num_devices=N_CORES)

    p1aug = nc.dram_tensor("p1aug", [20, n1], F16, kind="ExternalInput").ap()
    p2aug = nc.dram_tensor("p2aug", [20, n2], F16, kind="ExternalInput").ap()
    rowmin_d = nc.dram_tensor("rowmin", [P, n_it], F32, kind="ExternalOutput").ap()
    colmin_d = nc.dram_tensor("colmin", [P, n2 // P], F32, kind="ExternalOutput").ap()

    with tile.TileContext(nc) as tc, ExitStack() as ctx:
        singles = ctx.enter_context(tc.tile_pool(name="singles", bufs=1))
        psum_pool = ctx.enter_context(tc.tile_pool(name="psum", bufs=psum_bufs, space="PSUM"))
        d2c_pool = ctx.enter_context(tc.tile_pool(name="d2c", bufs=d2c_bufs))
        acc_pool = ctx.enter_context(tc.tile_pool(name="acc", bufs=2))

        lhs_sb = singles.tile([20, n1], F16)
        rhs_sb = singles.tile([20, n2], F16)
        # SWDGE (single queue) so the first matmul's wait fits the ISA's
        # sync-wait slots; HWDGE fans out to several queue semaphores.
        nc.gpsimd.dma_start(out=lhs_sb, in_=p1aug)
        nc.gpsimd.dma_start(out=rhs_sb, in_=p2aug)

        identity = singles.tile([P, P], F16)
        make_identity(nc, identity)

        colacc = singles.tile([P, n2], F16)
        rowmin_sb = singles.tile([P, n_it], F32)
        colmin_sb = singles.tile([P, n2 // P], F32)

        INF = 3.0e38  # finite +inf stand-in (orjson turns real inf into null)
        for rep in range(repeat):  # repeat>1: timing only (idempotent body)
          for it in range(n_it):
            lhs_slice = lhs_sb[:, it * P:(it + 1) * P]
            if use_ttr:
                rowpart = acc_pool.tile([P, n_jg], F32, tag="rowpart")
            else:
                rowacc = acc_pool.tile([P, gj], F16, tag="rowacc")
            for jg in range(n_jg):
                ps = psum_pool.tile([P, gj], F32, tag="mm")
                for g in range(nb):
                    nc.tensor.matmul(
                        ps[:, g * MM_N:(g + 1) * MM_N],
                        lhs_slice,
                        rhs_sb[:, jg * gj + g * MM_N: jg * gj + (g + 1) * MM_N],
                        start=True, stop=True,
                    )
                d2c = d2c_pool.tile([P, gj], F16, tag="d2c")
                nc.scalar.copy(out=d2c, in_=ps)

                cslice = colacc[:, jg * gj:(jg + 1) * gj]
                if use_ttr:
                    # Fused DVE pass #1: scratch = min(d2c, d2c) (= d2c) with
                    # accum_out = row-wise min of this group (init +inf).
                    # The accum must NOT read cslice — colacc holds minima of
                    # OTHER rows (previous i-blocks) and would contaminate.
                    scratch = d2c_pool.tile([P, gj], F16, tag="ttr_scratch")
                    nc.vector.tensor_tensor_reduce(
                        out=scratch, in0=d2c, in1=d2c, scale=1.0, scalar=INF,
                        op0=MIN, op1=MIN,
                        accum_out=rowpart[:, jg:jg + 1],
                    )
                    # DVE pass #2: column accumulator update.
                    if it == 0 and rep == 0:
                        nc.vector.tensor_copy(out=cslice, in_=d2c)
                    else:
                        nc.vector.tensor_tensor(out=cslice, in0=cslice,
                                                in1=d2c, op=MIN)
                else:
                    if jg == 0:
                        nc.vector.tensor_copy(out=rowacc, in_=d2c)
                    else:
                        nc.vector.tensor_tensor(out=rowacc, in0=rowacc,
                                                in1=d2c, op=MIN)
                    if it == 0:
                        nc.vector.tensor_copy(out=cslice, in_=d2c)
                    else:
                        nc.vector.tensor_tensor(out=cslice, in0=cslice,
                                                in1=d2c, op=MIN)

            if use_ttr:
                nc.vector.tensor_reduce(
                    out=rowmin_sb[:, it:it + 1], in_=rowpart,
                    axis=mybir.AxisListType.X, op=MIN,
                )
            else:
                # hierarchical fold halves tensor_reduce's 1x-rate element
                # count twice (tensor_tensor runs 2x on fp16 SBUF)
                h = gj // 2
                nc.vector.tensor_tensor(out=rowacc[:, :h], in0=rowacc[:, :h],
                                        in1=rowacc[:, h:], op=MIN)
                q = h // 2
                nc.vector.tensor_tensor(out=rowacc[:, :q], in0=rowacc[:, :q],
                                        in1=rowacc[:, q:h], op=MIN)
                nc.vector.tensor_reduce(
                    out=rowmin_sb[:, it:it + 1], in_=rowacc[:, :q],
                    axis=mybir.AxisListType.X, op=MIN,
                )

        # Column minima: cross-partition reduce via PE transpose.
        for jg in range(n_jg):
            pst = psum_pool.tile([P, n_tp, P], F16, tag="mm")
            for t in range(n_tp):
                nc.tensor.transpose(
                    pst[:, t],
                    colacc[:, jg * gj + t * P: jg * gj + (t + 1) * P],
                    identity,
                )
            nc.vector.tensor_reduce(
                out=colmin_sb[:, jg * n_tp:(jg + 1) * n_tp], in_=pst,
                axis=mybir.AxisListType.X, op=MIN,
            )

        nc.sync.dma_start(out=rowmin_d, in_=rowmin_sb)
        nc.sync.dma_start(out=colmin_d, in_=colmin_sb)

    _install_wait_splitter(nc)
    return nc


def _hilo(a: np.ndarray):
    """Split f32 rows into (hi, lo) fp16 pairs with a + 0 == hi + lo exactly
    representable: hi = fp16(a), lo = fp16(a - hi)."""
    hi = a.astype(np.float16)
    lo = (a - hi.astype(np.float32)).astype(np.float16)
    return hi, lo


def make_aug_inputs(points1: np.ndarray, points2: np.ndarray):
    """Host-side layout prep: augmented transposed hi/lo fp16 operands.

    d2 = sum_k A_k.B_k with A = [x1,y1,z1,1,sq1], B = [-2x2,-2y2,-2z2,sq2,1].
    K=20 fp16 rows compute (hiA+loA).(hiB+loB) = A.B up to f32 accumulation:
      lhsT rows: [hiA, hiA, loA, loA],  rhs rows: [hiB, loB, hiB, loB].
    """
    p1 = np.ascontiguousarray(points1, dtype=np.float32)
    p2 = np.ascontiguousarray(points2, dtype=np.float32)
    n1, n2 = p1.shape[0], p2.shape[0]
    sq1 = (p1 * p1).sum(axis=1)
    sq2 = (p2 * p2).sum(axis=1)
    ones2 = np.ones(n2, dtype=np.float32)
    B = np.stack([-2.0 * p2[:, 0], -2.0 * p2[:, 1], -2.0 * p2[:, 2], sq2, ones2])
    hiB, loB = _hilo(B)
    p2aug = np.ascontiguousarray(np.concatenate([hiB, loB, hiB, loB]))
    shard = n1 // N_CORES
    in_maps = []
    for c in range(N_CORES):
        s = slice(c * shard, (c + 1) * shard)
        p1c, sq1c = p1[s], sq1[s]
        ones1 = np.ones(shard, dtype=np.float32)
        A = np.stack([p1c[:, 0], p1c[:, 1], p1c[:, 2], ones1, sq1c])
        hiA, loA = _hilo(A)
        p1aug = np.ascontiguousarray(np.concatenate([hiA, hiA, loA, loA]))
        in_maps.append({"p1aug": p1aug, "p2aug": p2aug})
    return in_maps


def combine_outputs(results: list) -> np.ndarray:
    """Host-side unshard: gather per-core minima, final min/sqrt/sum."""
    rowmins = np.concatenate(
        [np.asarray(r["rowmin"]).T.reshape(-1) for r in results]
    )  # (n1,) squared distances
    colmin = np.stack(
        [np.asarray(r["colmin"]).T.reshape(-1) for r in results]
    ).min(axis=0)  # (n2,)
    total = (np.sqrt(np.maximum(rowmins, 0.0, dtype=np.float64)).sum()
             + np.sqrt(np.maximum(colmin, 0.0, dtype=np.float64)).sum())
    return np.float32(total)


class Runner:
    """Cached jitted 8-core executable (mirrors bass2jax.run_bass_via_pjrt's
    multi-core path, but reusable across calls without re-tracing)."""

    def __init__(self, nc, n_cores: int = N_CORES):
        import jax
        from jax.sharding import Mesh, NamedSharding, PartitionSpec

        try:
            from jax import shard_map
        except ImportError:
            from jax.experimental.shard_map import shard_map

        from concourse import bass2jax
        from concourse.bass2jax import _bass_exec_p, install_neuronx_cc_hook

        install_neuronx_cc_hook()
        self.jax = jax
        self.n_cores = n_cores
        pname = nc.partition_id_tensor.name if nc.partition_id_tensor else None
        self.in_names, self.out_names, self.out_avals, self.zero_outs = [], [], [], []
        for alloc in nc.m.functions[0].allocations:
            if not isinstance(alloc, mybir.MemoryLocationSet):
                continue
            name = alloc.memorylocations[0].name
            if alloc.kind == "ExternalInput":
                if name != pname:
                    self.in_names.append(name)
            elif alloc.kind == "ExternalOutput":
                shape = tuple(alloc.tensor_shape)
                dtype = mybir.dt.np(alloc.dtype)
                self.out_names.append(name)
                self.out_avals.append(jax.core.ShapedArray(shape, dtype))
                self.zero_outs.append(np.zeros(shape, dtype))
        n_params, n_outs = len(self.in_names), len(self.out_names)
        all_in = list(self.in_names) + self.out_names + ([pname] if pname else [])

        def _body(*args):
            operands = list(args)
            if pname is not None:
                operands.append(bass2jax.partition_id_tensor())
            return tuple(_bass_exec_p.bind(
                *operands, out_avals=tuple(self.out_avals),
                in_names=tuple(all_in), out_names=tuple(self.out_names),
                lowering_input_output_aliases=(),
                sim_require_finite=True, sim_require_nnan=True, nc=nc))

        devices = jax.devices()[:n_cores]
        mesh = Mesh(np.asarray(devices), ("core",))
        sm_kwargs = dict(mesh=mesh,
                         in_specs=(PartitionSpec("core"),) * (n_params + n_outs),
                         out_specs=(PartitionSpec("core"),) * n_outs)
        try:
            smapped = shard_map(_body, check_vma=False, **sm_kwargs)
        except TypeError:
            smapped = shard_map(_body, check_rep=False, **sm_kwargs)
        self.sharded = jax.jit(
            smapped,
            donate_argnums=tuple(range(n_params, n_params + n_outs)),
            keep_unused=True)
        self.sharding = NamedSharding(mesh, PartitionSpec("core"))

    def stage_inputs(self, in_maps):
        cat = [np.concatenate([np.asarray(in_maps[c][n])
                               for c in range(self.n_cores)], axis=0)
               for n in self.in_names]
        return [self.jax.device_put(a, self.sharding) for a in cat]

    def fresh_zeros(self):
        return [self.jax.device_put(
            np.zeros((self.n_cores * z.shape[0], *z.shape[1:]), z.dtype),
            self.sharding) for z in self.zero_outs]

    def run(self, in_maps):
        out_arrs = self.sharded(*self.stage_inputs(in_maps), *self.fresh_zeros())
        self.jax.block_until_ready(out_arrs)
        return self.unpack(out_arrs)

    def unpack(self, out_arrs):
        return [{n: np.asarray(out_arrs[i]).reshape(
                     self.n_cores, *self.out_avals[i].shape)[c]
                 for i, n in enumerate(self.out_names)}
                for c in range(self.n_cores)]


_CACHED = {}


def get_runner(n1_shard: int, n2: int, repeat: int = 1) -> Runner:
    key = (n1_shard, n2, repeat)
    if key not in _CACHED:
        _CACHED[key] = Runner(build_chamfer_bass(n1_shard, n2, repeat=repeat))
    return _CACHED[key]


def kernel(points1: np.ndarray, points2: np.ndarray) -> np.ndarray:
    p1 = np.asarray(points1)
    p2 = np.asarray(points2)
    runner = get_runner(p1.shape[0] // N_CORES, p2.shape[0])
    results = runner.run(make_aug_inputs(p1, p2))
    return combine_outputs(results)


def _numpy_ref(p1: np.ndarray, p2: np.ndarray, chunk: int = 1024) -> float:
    """Chunked numpy chamfer reference (f32 matmul expansion like the oracle)."""
    sq1 = (p1 * p1).sum(1)
    sq2 = (p2 * p2).sum(1)
    rowmin = np.full(p1.shape[0], np.inf, np.float32)
    colmin = np.full(p2.shape[0], np.inf, np.float32)
    for s in range(0, p1.shape[0], chunk):
        d2 = (sq1[s:s + chunk, None] + sq2[None, :]
              - 2.0 * (p1[s:s + chunk] @ p2.T))
        rowmin[s:s + chunk] = d2.min(1)
        np.minimum(colmin, d2.min(0), out=colmin)
    return float(np.sqrt(np.maximum(rowmin, 0)).sum()
                 + np.sqrt(np.maximum(colmin, 0)).sum())


if __name__ == "__main__":
    rng = np.random.default_rng(0)
    p1 = rng.standard_normal((N1_FULL, 3), dtype=np.float32)
    p2 = rng.standard_normal((N2_FULL, 3), dtype=np.float32)
    out = kernel(p1, p2)
    ref = _numpy_ref(p1, p2)
    print("kernel:", out, "ref:", ref, "rel:", abs(out - ref) / abs(ref))


# revision 22
# speedup vs baseline: 284.1242x; 265.2960x over previous
"""Chamfer distance kernel for Trainium2, 8 NeuronCores.

Problem: points1 (16384,3) f32, points2 (16384,3) f32.
  dist = cdist(points1, points2); out = sum(min(dist,axis=1)) + sum(min(dist,axis=0))

Strategy:
  - Shard points1 rows across the 8 cores (2048 rows each); points2 replicated.
  - d2[i,j] = ||p1_i||^2 + ||p2_j||^2 - 2 p1_i.p2_j is produced directly by an
    augmented matmul on TensorE with A = [x1, y1, z1, 1, sq1] against
    B = [-2x2, -2y2, -2z2, sq2, 1]. fp32 matmuls stream at 1/4 PE rate, so
    A and B are hi/lo-split into fp16 and fed as one K=20 fp16 matmul
    (hiA+loA).(hiB+loB) — exact products, f32 PSUM accumulation, full rate.
  - ScalarE casts each PSUM d2 block to fp16 in SBUF; VectorE accumulates
    row minima and column minima with fp16 tensor_tensor(min) at 2x rate.
    (min commutes with the monotone sqrt, so sqrt is applied at the end to
    the 16384+16384 minima only, on host, in f64.)
  - Column minima need a cross-partition reduce: PE-transpose (fp16) of the
    column accumulator + free-axis reduce.
  - Per-core partial outputs (row minima of its row block, column-minima
    partial over its rows) are combined on host: min across cores, sqrt, sum.
"""

import sys
from contextlib import ExitStack

import numpy as np

sys.path.insert(0, "/opt/trn_rl_repo")

import concourse.bass as bass  # noqa: E402
import concourse.tile as tile  # noqa: E402
from concourse import mybir  # noqa: E402
from concourse.masks import make_identity  # noqa: E402

F32 = mybir.dt.float32
F16 = mybir.dt.float16
MIN = mybir.AluOpType.min


def _split_multi_waits(bir: dict, max_waits: int = 1) -> dict:
    """Hoist extra sync waits into standalone EventSemaphore instructions.

    The walrus in this container encodes at most one sync-wait command per
    instruction (raw bass emits standalone EventSemaphore waits for the same
    reason); Tile attaches every required wait to the consuming instruction.
    Splitting preserves semantics exactly: the engine stalls at the hoisted
    wait(s), then executes the op with the remaining wait.
    """
    for fn in bir["functions"]:
        for blk in fn["blocks"]:
            new = []
            for inst in blk["instructions"]:
                si = inst.get("sync_info")
                waits = (si or {}).get("on_wait") or []
                if si is not None and len(waits) > max_waits:
                    for k, w in enumerate(waits[:-max_waits]):
                        new.append({
                            "debug": inst.get("debug", 0),
                            "engine": inst["engine"],
                            "ins": [], "outs": [],
                            "name": f'{inst["name"]}-hw{k}',
                            "opcode": "EventSemaphore",
                            "sync_info": {"on_update": [], "on_wait": [w]},
                        })
                    si["on_wait"] = waits[-max_waits:]
                new.append(inst)
            blk["instructions"] = new
    return bir


def _install_wait_splitter(nc: bass.Bass) -> None:
    import orjson

    orig = nc.to_json_bytes

    def patched() -> bytes:
        return orjson.dumps(_split_multi_waits(orjson.loads(orig())))

    nc.to_json_bytes = patched

N_CORES = 8
N1_FULL = 16384
N2_FULL = 16384
P = 128  # partitions
MM_N = 512  # matmul free dim (one PSUM bank of f32)


def build_chamfer_bass(n1: int, n2: int, gj: int = 2048, repeat: int = 1,
                       use_ttr: bool = False, d2c_bufs: int = 3,
                       psum_bufs: int = 2) -> bass.Bass:
    # use_ttr=True (fused TensorTensorReduce rowmin) validates in CoreSim but
    # this container's walrus rejects the custom DVE ISA op ("ISA wrong
    # length"), so it stays off.
    """Build the per-core Bass program.

    fp32 matmuls stream at 1/4 rate on the PE, so the K=5 augmented operands
    are split hi/lo into fp16 and fed as a K=20 fp16 matmul computing
    (hiA+loA).(hiB+loB) exactly (fp16 products are exact in the f32
    accumulator), i.e. fp32-level d2 at full PE streaming rate.

    Per-core inputs:
      p1aug [20, n1] f16 : rows [hiA, hiA, loA, loA], A = [x1, y1, z1, 1, sq1]
      p2aug [20, n2] f16 : rows [hiB, loB, hiB, loB], B = [-2x2, -2y2, -2z2, sq2, 1]
    Per-core outputs:
      rowmin [128, n1//128] f32 : rowmin[p, it] = min_j d2[it*128+p, j]
      colmin [128, n2//128] f32 : colmin[p, c]  = min_{i in core rows} d2[i, c*128+p]
    """
    assert n1 % P == 0 and n2 % gj == 0 and gj % MM_N == 0
    n_it = n1 // P
    n_jg = n2 // gj
    nb = gj // MM_N
    n_tp = gj // P  # transposes per j-group

    nc = bass.Bass("TRN2", target_bir_lowering=False, debug=False,
                   num_devices=N_CORES)

    p1aug = nc.dram_tensor("p1aug", [20, n1], F16, kind="ExternalInput").ap()
    p2aug = nc.dram_tensor("p2aug", [20, n2], F16, kind="ExternalInput").ap()
    rowmin_d = nc.dram_tensor("rowmin", [P, n_it], F32, kind="ExternalOutput").ap()
    colmin_d = nc.dram_tensor("colmin", [P, n2 // P], F32, kind="ExternalOutput").ap()

    with tile.TileContext(nc) as tc, ExitStack() as ctx:
        singles = ctx.enter_context(tc.tile_pool(name="singles", bufs=1))
        psum_pool = ctx.enter_context(tc.tile_pool(name="psum", bufs=psum_bufs, space="PSUM"))
        d2c_pool = ctx.enter_context(tc.tile_pool(name="d2c", bufs=d2c_bufs))
        acc_pool = ctx.enter_context(tc.tile_pool(name="acc", bufs=2))

        lhs_sb = singles.tile([20, n1], F16)
        rhs_sb = singles.tile([20, n2], F16)
        # SWDGE (single queue) so the first matmul's wait fits the ISA's
        # sync-wait slots; HWDGE fans out to several queue semaphores.
        nc.gpsimd.dma_start(out=lhs_sb, in_=p1aug)
        nc.gpsimd.dma_start(out=rhs_sb, in_=p2aug)

        identity = singles.tile([P, P], F16)
        make_identity(nc, identity)

        colacc = singles.tile([P, n2], F16)
        rowmin_sb = singles.tile([P, n_it], F32)
        colmin_sb = singles.tile([P, n2 // P], F32)

        INF = 3.0e38  # finite +inf stand-in (orjson turns real inf into null)
        for rep in range(repeat):  # repeat>1: timing only (idempotent body)
          for it in range(n_it):
            lhs_slice = lhs_sb[:, it * P:(it + 1) * P]
            if use_ttr:
                rowpart = acc_pool.tile([P, n_jg], F32, tag="rowpart")
            else:
                rowacc = acc_pool.tile([P, gj], F16, tag="rowacc")
            for jg in range(n_jg):
                ps = psum_pool.tile([P, gj], F32, tag="mm")
                for g in range(nb):
                    nc.tensor.matmul(
                        ps[:, g * MM_N:(g + 1) * MM_N],
                        lhs_slice,
                        rhs_sb[:, jg * gj + g * MM_N: jg * gj + (g + 1) * MM_N],
                        start=True, stop=True,
                    )
                d2c = d2c_pool.tile([P, gj], F16, tag="d2c")
                nc.scalar.copy(out=d2c, in_=ps)

                cslice = colacc[:, jg * gj:(jg + 1) * gj]
                if use_ttr:
                    # Fused DVE pass #1: scratch = min(d2c, d2c) (= d2c) with
                    # accum_out = row-wise min of this group (init +inf).
                    # The accum must NOT read cslice — colacc holds minima of
                    # OTHER rows (previous i-blocks) and would contaminate.
                    scratch = d2c_pool.tile([P, gj], F16, tag="ttr_scratch")
                    nc.vector.tensor_tensor_reduce(
                        out=scratch, in0=d2c, in1=d2c, scale=1.0, scalar=INF,
                        op0=MIN, op1=MIN,
                        accum_out=rowpart[:, jg:jg + 1],
                    )
                    # DVE pass #2: column accumulator update.
                    if it == 0 and rep == 0:
                        nc.vector.tensor_copy(out=cslice, in_=d2c)
                    else:
                        nc.vector.tensor_tensor(out=cslice, in0=cslice,
                                                in1=d2c, op=MIN)
                else:
                    if jg == 0:
                        nc.vector.tensor_copy(out=rowacc, in_=d2c)
                    else:
                        nc.vector.tensor_tensor(out=rowacc, in0=rowacc,
                                                in1=d2c, op=MIN)
                    if it == 0:
                        nc.vector.tensor_copy(out=cslice, in_=d2c)
                    else:
                        nc.vector.tensor_tensor(out=cslice, in0=cslice,
                                                in1=d2c, op=MIN)

            if use_ttr:
                nc.vector.tensor_reduce(
                    out=rowmin_sb[:, it:it + 1], in_=rowpart,
                    axis=mybir.AxisListType.X, op=MIN,
                )
            else:
                # hierarchical fold halves tensor_reduce's 1x-rate element
                # count twice (tensor_tensor runs 2x on fp16 SBUF)
                h = gj // 2
                nc.vector.tensor_tensor(out=rowacc[:, :h], in0=rowacc[:, :h],
                                        in1=rowacc[:, h:], op=MIN)
                q = h // 2
                nc.vector.tensor_tensor(out=rowacc[:, :q], in0=rowacc[:, :q],
                                        in1=rowacc[:, q:h], op=MIN)
                nc.vector.tensor_reduce(
                    out=rowmin_sb[:, it:it + 1], in_=rowacc[:, :q],
                    axis=mybir.AxisListType.X, op=MIN,
                )

        # Column minima: cross-partition reduce via PE transpose.
        for jg in range(n_jg):
            pst = psum_pool.tile([P, n_tp, P], F16, tag="mm")
            for t in range(n_tp):
                nc.tensor.transpose(
                    pst[:, t],
                    colacc[:, jg * gj + t * P: jg * gj + (t + 1) * P],
                    identity,
                )
            nc.vector.tensor_reduce(
                out=colmin_sb[:, jg * n_tp:(jg + 1) * n_tp], in_=pst,
                axis=mybir.AxisListType.X, op=MIN,
            )

        nc.sync.dma_start(out=rowmin_d, in_=rowmin_sb)
        nc.sync.dma_start(out=colmin_d, in_=colmin_sb)

    _install_wait_splitter(nc)
    return nc


def _hilo(a: np.ndarray):
    """Split f32 rows into (hi, lo) fp16 pairs with a + 0 == hi + lo exactly
    representable: hi = fp16(a), lo = fp16(a - hi)."""
    hi = a.astype(np.float16)
    lo = (a - hi.astype(np.float32)).astype(np.float16)
    return hi, lo


def make_aug_inputs(points1: np.ndarray, points2: np.ndarray):
    """Host-side layout prep: augmented transposed hi/lo fp16 operands.

    d2 = sum_k A_k.B_k with A = [x1,y1,z1,1,sq1], B = [-2x2,-2y2,-2z2,sq2,1].
    K=20 fp16 rows compute (hiA+loA).(hiB+loB) = A.B up to f32 accumulation:
      lhsT rows: [hiA, hiA, loA, loA],  rhs rows: [hiB, loB, hiB, loB].
    """
    p1 = np.ascontiguousarray(points1, dtype=np.float32)
    p2 = np.ascontiguousarray(points2, dtype=np.float32)
    n1, n2 = p1.shape[0], p2.shape[0]
    sq1 = (p1 * p1).sum(axis=1)
    sq2 = (p2 * p2).sum(axis=1)
    ones2 = np.ones(n2, dtype=np.float32)
    B = np.stack([-2.0 * p2[:, 0], -2.0 * p2[:, 1], -2.0 * p2[:, 2], sq2, ones2])
    hiB, loB = _hilo(B)
    p2aug = np.ascontiguousarray(np.concatenate([hiB, loB, hiB, loB]))
    shard = n1 // N_CORES
    in_maps = []
    for c in range(N_CORES):
        s = slice(c * shard, (c + 1) * shard)
        p1c, sq1c = p1[s], sq1[s]
        ones1 = np.ones(shard, dtype=np.float32)
        A = np.stack([p1c[:, 0], p1c[:, 1], p1c[:, 2], ones1, sq1c])
        hiA, loA = _hilo(A)
        p1aug = np.ascontiguousarray(np.concatenate([hiA, hiA, loA, loA]))
        in_maps.append({"p1aug": p1aug, "p2aug": p2aug})
    return in_maps


def combine_outputs(results: list) -> np.ndarray:
    """Host-side unshard: gather per-core minima, final min/sqrt/sum."""
    rowmins = np.concatenate(
        [np.asarray(r["rowmin"]).T.reshape(-1) for r in results]
    )  # (n1,) squared distances
    colmin = np.stack(
        [np.asarray(r["colmin"]).T.reshape(-1) for r in results]
    ).min(axis=0)  # (n2,)
    total = (np.sqrt(np.maximum(rowmins, 0.0, dtype=np.float64)).sum()
             + np.sqrt(np.maximum(colmin, 0.0, dtype=np.float64)).sum())
    return np.asarray(total, dtype=np.float32)


class Runner:
    """Cached jitted 8-core executable (mirrors bass2jax.run_bass_via_pjrt's
    multi-core path, but reusable across calls without re-tracing)."""

    def __init__(self, nc, n_cores: int = N_CORES):
        import jax
        from jax.sharding import Mesh, NamedSharding, PartitionSpec

        try:
            from jax import shard_map
        except ImportError:
            from jax.experimental.shard_map import shard_map

        from concourse import bass2jax
        from concourse.bass2jax import _bass_exec_p, install_neuronx_cc_hook

        install_neuronx_cc_hook()
        self.jax = jax
        self.n_cores = n_cores
        pname = nc.partition_id_tensor.name if nc.partition_id_tensor else None
        self.in_names, self.out_names, self.out_avals, self.zero_outs = [], [], [], []
        for alloc in nc.m.functions[0].allocations:
            if not isinstance(alloc, mybir.MemoryLocationSet):
                continue
            name = alloc.memorylocations[0].name
            if alloc.kind == "ExternalInput":
                if name != pname:
                    self.in_names.append(name)
            elif alloc.kind == "ExternalOutput":
                shape = tuple(alloc.tensor_shape)
                dtype = mybir.dt.np(alloc.dtype)
                self.out_names.append(name)
                self.out_avals.append(jax.core.ShapedArray(shape, dtype))
                self.zero_outs.append(np.zeros(shape, dtype))
        n_params, n_outs = len(self.in_names), len(self.out_names)
        all_in = list(self.in_names) + self.out_names + ([pname] if pname else [])

        def _body(*args):
            operands = list(args)
            if pname is not None:
                operands.append(bass2jax.partition_id_tensor())
            return tuple(_bass_exec_p.bind(
                *operands, out_avals=tuple(self.out_avals),
                in_names=tuple(all_in), out_names=tuple(self.out_names),
                lowering_input_output_aliases=(),
                sim_require_finite=True, sim_require_nnan=True, nc=nc))

        devices = jax.devices()[:n_cores]
        mesh = Mesh(np.asarray(devices), ("core",))
        sm_kwargs = dict(mesh=mesh,
                         in_specs=(PartitionSpec("core"),) * (n_params + n_outs),
                         out_specs=(PartitionSpec("core"),) * n_outs)
        try:
            smapped = shard_map(_body, check_vma=False, **sm_kwargs)
        except TypeError:
            smapped = shard_map(_body, check_rep=False, **sm_kwargs)
        self.sharded = jax.jit(
            smapped,
            donate_argnums=tuple(range(n_params, n_params + n_outs)),
            keep_unused=True)
        self.sharding = NamedSharding(mesh, PartitionSpec("core"))

    def stage_inputs(self, in_maps):
        cat = [np.concatenate([np.asarray(in_maps[c][n])
                               for c in range(self.n_cores)], axis=0)
               for n in self.in_names]
        return [self.jax.device_put(a, self.sharding) for a in cat]

    def fresh_zeros(self):
        return [self.jax.device_put(
            np.zeros((self.n_cores * z.shape[0], *z.shape[1:]), z.dtype),
            self.sharding) for z in self.zero_outs]

    def run(self, in_maps):
        out_arrs = self.sharded(*self.stage_inputs(in_maps), *self.fresh_zeros())
        self.jax.block_until_ready(out_arrs)
        return self.unpack(out_arrs)

    def unpack(self, out_arrs):
        return [{n: np.asarray(out_arrs[i]).reshape(
                     self.n_cores, *self.out_avals[i].shape)[c]
                 for i, n in enumerate(self.out_names)}
                for c in range(self.n_cores)]


_CACHED = {}


def get_runner(n1_shard: int, n2: int, repeat: int = 1) -> Runner:
    key = (n1_shard, n2, repeat)
    if key not in _CACHED:
        _CACHED[key] = Runner(build_chamfer_bass(n1_shard, n2, repeat=repeat))
    return _CACHED[key]


def kernel(points1: np.ndarray, points2: np.ndarray) -> np.ndarray:
    p1 = np.asarray(points1)
    p2 = np.asarray(points2)
    runner = get_runner(p1.shape[0] // N_CORES, p2.shape[0])
    results = runner.run(make_aug_inputs(p1, p2))
    return combine_outputs(results)


def _numpy_ref(p1: np.ndarray, p2: np.ndarray, chunk: int = 1024) -> float:
    """Chunked numpy chamfer reference (f32 matmul expansion like the oracle)."""
    sq1 = (p1 * p1).sum(1)
    sq2 = (p2 * p2).sum(1)
    rowmin = np.full(p1.shape[0], np.inf, np.float32)
    colmin = np.full(p2.shape[0], np.inf, np.float32)
    for s in range(0, p1.shape[0], chunk):
        d2 = (sq1[s:s + chunk, None] + sq2[None, :]
              - 2.0 * (p1[s:s + chunk] @ p2.T))
        rowmin[s:s + chunk] = d2.min(1)
        np.minimum(colmin, d2.min(0), out=colmin)
    return float(np.sqrt(np.maximum(rowmin, 0)).sum()
                 + np.sqrt(np.maximum(colmin, 0)).sum())


if __name__ == "__main__":
    rng = np.random.default_rng(0)
    p1 = rng.standard_normal((N1_FULL, 3), dtype=np.float32)
    p2 = rng.standard_normal((N2_FULL, 3), dtype=np.float32)
    out = kernel(p1, p2)
    ref = _numpy_ref(p1, p2)
    print("kernel:", out, "ref:", ref, "rel:", abs(out - ref) / abs(ref))


# revision 27
# speedup vs baseline: 313.6866x; 1.1040x over previous
"""Chamfer distance kernel for Trainium2, 8 NeuronCores.

Problem: points1 (16384,3) f32, points2 (16384,3) f32.
  dist = cdist(points1, points2); out = sum(min(dist,axis=1)) + sum(min(dist,axis=0))

Strategy:
  - Shard points1 rows across the 8 cores (2048 rows each); points2 replicated.
  - d2[i,j] = ||p1_i||^2 + ||p2_j||^2 - 2 p1_i.p2_j is produced directly by an
    augmented matmul on TensorE with A = [x1, y1, z1, 1, sq1] against
    B = [-2x2, -2y2, -2z2, sq2, 1]. fp32 matmuls stream at 1/4 PE rate, so
    A and B are hi/lo-split into fp16 and fed as one K=20 fp16 matmul
    (hiA+loA).(hiB+loB) — exact products, f32 PSUM accumulation, full rate.
  - ScalarE casts each PSUM d2 block to fp16 in SBUF; VectorE accumulates
    row minima and column minima with fp16 tensor_tensor(min) at 2x rate.
    (min commutes with the monotone sqrt, so sqrt is applied at the end to
    the 16384+16384 minima only, on host, in f64.)
  - Column minima need a cross-partition reduce: PE-transpose (fp16) of the
    column accumulator + free-axis reduce.
  - Per-core partial outputs (row minima of its row block, column-minima
    partial over its rows) are combined on host: min across cores, sqrt, sum.
"""

import sys
from contextlib import ExitStack

import numpy as np

sys.path.insert(0, "/opt/trn_rl_repo")

import concourse.bass as bass  # noqa: E402
import concourse.tile as tile  # noqa: E402
from concourse import mybir  # noqa: E402
from concourse.masks import make_identity  # noqa: E402

F32 = mybir.dt.float32
F16 = mybir.dt.float16
MIN = mybir.AluOpType.min


def _split_multi_waits(bir: dict, max_waits: int = 1) -> dict:
    """Hoist extra sync waits into standalone EventSemaphore instructions.

    The walrus in this container encodes at most one sync-wait command per
    instruction (raw bass emits standalone EventSemaphore waits for the same
    reason); Tile attaches every required wait to the consuming instruction.
    Splitting preserves semantics exactly: the engine stalls at the hoisted
    wait(s), then executes the op with the remaining wait.
    """
    for fn in bir["functions"]:
        for blk in fn["blocks"]:
            new = []
            for inst in blk["instructions"]:
                si = inst.get("sync_info")
                waits = (si or {}).get("on_wait") or []
                if si is not None and len(waits) > max_waits:
                    for k, w in enumerate(waits[:-max_waits]):
                        new.append({
                            "debug": inst.get("debug", 0),
                            "engine": inst["engine"],
                            "ins": [], "outs": [],
                            "name": f'{inst["name"]}-hw{k}',
                            "opcode": "EventSemaphore",
                            "sync_info": {"on_update": [], "on_wait": [w]},
                        })
                    si["on_wait"] = waits[-max_waits:]
                new.append(inst)
            blk["instructions"] = new
    return bir


def _install_wait_splitter(nc: bass.Bass) -> None:
    import orjson

    orig = nc.to_json_bytes

    def patched() -> bytes:
        return orjson.dumps(_split_multi_waits(orjson.loads(orig())))

    nc.to_json_bytes = patched

N_CORES = 8
N1_FULL = 16384
N2_FULL = 16384
P = 128  # partitions
MM_N = 512  # matmul free dim (one PSUM bank of f32)


def build_chamfer_bass(n1: int, n2: int, gj: int = 2048, repeat: int = 1,
                       use_ttr: bool = False, d2c_bufs: int = 3,
                       psum_bufs: int = 2) -> bass.Bass:
    # use_ttr=True (fused TensorTensorReduce rowmin) validates in CoreSim but
    # this container's walrus rejects the custom DVE ISA op ("ISA wrong
    # length"), so it stays off.
    """Build the per-core Bass program.

    fp32 matmuls stream at 1/4 rate on the PE, so the K=5 augmented operands
    are split hi/lo into fp16 and fed as a K=20 fp16 matmul computing
    (hiA+loA).(hiB+loB) exactly (fp16 products are exact in the f32
    accumulator), i.e. fp32-level d2 at full PE streaming rate.

    Per-core inputs:
      p1aug [20, n1] f16 : rows [hiA, hiA, loA, loA], A = [x1, y1, z1, 1, sq1]
      p2aug [20, n2] f16 : rows [hiB, loB, hiB, loB], B = [-2x2, -2y2, -2z2, sq2, 1]
    Per-core outputs:
      rowmin [128, n1//128] f32 : rowmin[p, it] = min_j d2[it*128+p, j]
      colmin [128, n2//128] f32 : colmin[p, c]  = min_{i in core rows} d2[i, c*128+p]
    """
    assert n1 % P == 0 and n2 % gj == 0 and gj % MM_N == 0
    n_it = n1 // P
    n_jg = n2 // gj
    nb = gj // MM_N
    n_tp = gj // P  # transposes per j-group

    nc = bass.Bass("TRN2", target_bir_lowering=False, debug=False,
                   num_devices=N_CORES)

    p1aug = nc.dram_tensor("p1aug", [20, n1], F16, kind="ExternalInput").ap()
    p2aug = nc.dram_tensor("p2aug", [20, n2], F16, kind="ExternalInput").ap()
    rowmin_d = nc.dram_tensor("rowmin", [P, n_it], F32, kind="ExternalOutput").ap()
    colmin_d = nc.dram_tensor("colmin", [P, n2 // P], F32, kind="ExternalOutput").ap()

    with tile.TileContext(nc) as tc, ExitStack() as ctx:
        singles = ctx.enter_context(tc.tile_pool(name="singles", bufs=1))
        psum_pool = ctx.enter_context(tc.tile_pool(name="psum", bufs=psum_bufs, space="PSUM"))
        d2c_pool = ctx.enter_context(tc.tile_pool(name="d2c", bufs=d2c_bufs))
        acc_pool = ctx.enter_context(tc.tile_pool(name="acc", bufs=2))

        lhs_sb = singles.tile([20, n1], F16)
        rhs_sb = singles.tile([20, n2], F16)
        # SWDGE (single queue) so the first matmul's wait fits the ISA's
        # sync-wait slots; HWDGE fans out to several queue semaphores.
        nc.gpsimd.dma_start(out=lhs_sb, in_=p1aug)
        nc.gpsimd.dma_start(out=rhs_sb, in_=p2aug)

        identity = singles.tile([P, P], F16)
        make_identity(nc, identity)

        colacc = singles.tile([P, n2], F16)
        rowmin_sb = singles.tile([P, n_it], F32)
        colmin_sb = singles.tile([P, n2 // P], F32)

        INF = 3.0e38  # finite +inf stand-in (orjson turns real inf into null)
        for rep in range(repeat):  # repeat>1: timing only (idempotent body)
          for it in range(n_it):
            lhs_slice = lhs_sb[:, it * P:(it + 1) * P]
            if use_ttr:
                rowpart = acc_pool.tile([P, n_jg], F32, tag="rowpart")
            else:
                rowacc = acc_pool.tile([P, gj], F16, tag="rowacc")
            d2cp = None
            for jg in range(n_jg):
                ps = psum_pool.tile([P, gj], F32, tag="mm")
                for g in range(nb):
                    nc.tensor.matmul(
                        ps[:, g * MM_N:(g + 1) * MM_N],
                        lhs_slice,
                        rhs_sb[:, jg * gj + g * MM_N: jg * gj + (g + 1) * MM_N],
                        start=True, stop=True,
                    )
                # d2c tiles span PAIRS of j-groups so the colacc min below can
                # run one FD=2*gj op (per-op fixed cost amortized 2x)
                if jg % 2 == 0:
                    d2cp = d2c_pool.tile([P, 2 * gj], F16, tag="d2c")
                d2c = d2cp[:, (jg % 2) * gj:(jg % 2 + 1) * gj]
                nc.scalar.copy(out=d2c, in_=ps)

                cslice = colacc[:, jg * gj:(jg + 1) * gj]
                if use_ttr:
                    # Fused DVE pass #1: scratch = min(d2c, d2c) (= d2c) with
                    # accum_out = row-wise min of this group (init +inf).
                    # The accum must NOT read cslice — colacc holds minima of
                    # OTHER rows (previous i-blocks) and would contaminate.
                    scratch = d2c_pool.tile([P, gj], F16, tag="ttr_scratch")
                    nc.vector.tensor_tensor_reduce(
                        out=scratch, in0=d2c, in1=d2c, scale=1.0, scalar=INF,
                        op0=MIN, op1=MIN,
                        accum_out=rowpart[:, jg:jg + 1],
                    )
                    # DVE pass #2: column accumulator update.
                    if it == 0 and rep == 0:
                        nc.vector.tensor_copy(out=cslice, in_=d2c)
                    else:
                        nc.vector.tensor_tensor(out=cslice, in0=cslice,
                                                in1=d2c, op=MIN)
                else:
                    if jg == 0:
                        nc.vector.tensor_copy(out=rowacc, in_=d2c)
                    else:
                        nc.vector.tensor_tensor(out=rowacc, in0=rowacc,
                                                in1=d2c, op=MIN)
                    if jg % 2 == 1:  # both halves of the pair are ready
                        cpair = colacc[:, (jg - 1) * gj:(jg + 1) * gj]
                        if it == 0:
                            nc.vector.tensor_copy(out=cpair, in_=d2cp)
                        else:
                            nc.vector.tensor_tensor(out=cpair, in0=cpair,
                                                    in1=d2cp, op=MIN)

            if use_ttr:
                nc.vector.tensor_reduce(
                    out=rowmin_sb[:, it:it + 1], in_=rowpart,
                    axis=mybir.AxisListType.X, op=MIN,
                )
            else:
                # hierarchical fold (2x-rate tensor_tensor) shrinks the
                # 1x-rate tensor_reduce's element count 8-fold
                w = gj
                while w > 256:
                    h = w // 2
                    nc.vector.tensor_tensor(out=rowacc[:, :h],
                                            in0=rowacc[:, :h],
                                            in1=rowacc[:, h:w], op=MIN)
                    w = h
                nc.vector.tensor_reduce(
                    out=rowmin_sb[:, it:it + 1], in_=rowacc[:, :w],
                    axis=mybir.AxisListType.X, op=MIN,
                )

        # Column minima: cross-partition reduce via PE transpose. Fold the
        # transposed tile's inner (i-part) axis 128->64 with a 2x-rate
        # tensor_tensor first; tensor_reduce only runs at 1x.
        for jg in range(n_jg):
            pst = psum_pool.tile([P, n_tp, P], F16, tag="mm")
            for t in range(n_tp):
                nc.tensor.transpose(
                    pst[:, t],
                    colacc[:, jg * gj + t * P: jg * gj + (t + 1) * P],
                    identity,
                )
            # only one PSUM operand is allowed per DVE op, so stage the
            # transposed tile into SBUF on the otherwise-idle ScalarE first
            cmt = acc_pool.tile([P, n_tp, P], F16, tag="cmt")
            nc.scalar.copy(out=cmt, in_=pst)
            cmf = acc_pool.tile([P, n_tp, P // 2], F16, tag="cmf")
            nc.vector.tensor_tensor(out=cmf, in0=cmt[:, :, :P // 2],
                                    in1=cmt[:, :, P // 2:], op=MIN)
            nc.vector.tensor_reduce(
                out=colmin_sb[:, jg * n_tp:(jg + 1) * n_tp], in_=cmf,
                axis=mybir.AxisListType.X, op=MIN,
            )

        nc.sync.dma_start(out=rowmin_d, in_=rowmin_sb)
        nc.sync.dma_start(out=colmin_d, in_=colmin_sb)

    _install_wait_splitter(nc)
    return nc


def _hilo(a: np.ndarray):
    """Split f32 rows into (hi, lo) fp16 pairs with a + 0 == hi + lo exactly
    representable: hi = fp16(a), lo = fp16(a - hi)."""
    hi = a.astype(np.float16)
    lo = (a - hi.astype(np.float32)).astype(np.float16)
    return hi, lo


def make_aug_inputs(points1: np.ndarray, points2: np.ndarray):
    """Host-side layout prep: augmented transposed hi/lo fp16 operands.

    d2 = sum_k A_k.B_k with A = [x1,y1,z1,1,sq1], B = [-2x2,-2y2,-2z2,sq2,1].
    K=20 fp16 rows compute (hiA+loA).(hiB+loB) = A.B up to f32 accumulation:
      lhsT rows: [hiA, hiA, loA, loA],  rhs rows: [hiB, loB, hiB, loB].
    """
    p1 = np.ascontiguousarray(points1, dtype=np.float32)
    p2 = np.ascontiguousarray(points2, dtype=np.float32)
    n1, n2 = p1.shape[0], p2.shape[0]
    sq1 = (p1 * p1).sum(axis=1)
    sq2 = (p2 * p2).sum(axis=1)
    ones2 = np.ones(n2, dtype=np.float32)
    B = np.stack([-2.0 * p2[:, 0], -2.0 * p2[:, 1], -2.0 * p2[:, 2], sq2, ones2])
    hiB, loB = _hilo(B)
    p2aug = np.ascontiguousarray(np.concatenate([hiB, loB, hiB, loB]))
    shard = n1 // N_CORES
    in_maps = []
    for c in range(N_CORES):
        s = slice(c * shard, (c + 1) * shard)
        p1c, sq1c = p1[s], sq1[s]
        ones1 = np.ones(shard, dtype=np.float32)
        A = np.stack([p1c[:, 0], p1c[:, 1], p1c[:, 2], ones1, sq1c])
        hiA, loA = _hilo(A)
        p1aug = np.ascontiguousarray(np.concatenate([hiA, hiA, loA, loA]))
        in_maps.append({"p1aug": p1aug, "p2aug": p2aug})
    return in_maps


def combine_outputs(results: list) -> np.ndarray:
    """Host-side unshard: gather per-core minima, final min/sqrt/sum."""
    rowmins = np.concatenate(
        [np.asarray(r["rowmin"]).T.reshape(-1) for r in results]
    )  # (n1,) squared distances
    colmin = np.stack(
        [np.asarray(r["colmin"]).T.reshape(-1) for r in results]
    ).min(axis=0)  # (n2,)
    total = (np.sqrt(np.maximum(rowmins, 0.0, dtype=np.float64)).sum()
             + np.sqrt(np.maximum(colmin, 0.0, dtype=np.float64)).sum())
    return np.asarray(total, dtype=np.float32)


class Runner:
    """Cached jitted 8-core executable (mirrors bass2jax.run_bass_via_pjrt's
    multi-core path, but reusable across calls without re-tracing)."""

    def __init__(self, nc, n_cores: int = N_CORES):
        import jax
        from jax.sharding import Mesh, NamedSharding, PartitionSpec

        try:
            from jax import shard_map
        except ImportError:
            from jax.experimental.shard_map import shard_map

        from concourse import bass2jax
        from concourse.bass2jax import _bass_exec_p, install_neuronx_cc_hook

        install_neuronx_cc_hook()
        self.jax = jax
        self.n_cores = n_cores
        pname = nc.partition_id_tensor.name if nc.partition_id_tensor else None
        self.in_names, self.out_names, self.out_avals, self.zero_outs = [], [], [], []
        for alloc in nc.m.functions[0].allocations:
            if not isinstance(alloc, mybir.MemoryLocationSet):
                continue
            name = alloc.memorylocations[0].name
            if alloc.kind == "ExternalInput":
                if name != pname:
                    self.in_names.append(name)
            elif alloc.kind == "ExternalOutput":
                shape = tuple(alloc.tensor_shape)
                dtype = mybir.dt.np(alloc.dtype)
                self.out_names.append(name)
                self.out_avals.append(jax.core.ShapedArray(shape, dtype))
                self.zero_outs.append(np.zeros(shape, dtype))
        n_params, n_outs = len(self.in_names), len(self.out_names)
        all_in = list(self.in_names) + self.out_names + ([pname] if pname else [])

        def _body(*args):
            operands = list(args)
            if pname is not None:
                operands.append(bass2jax.partition_id_tensor())
            return tuple(_bass_exec_p.bind(
                *operands, out_avals=tuple(self.out_avals),
                in_names=tuple(all_in), out_names=tuple(self.out_names),
                lowering_input_output_aliases=(),
                sim_require_finite=True, sim_require_nnan=True, nc=nc))

        devices = jax.devices()[:n_cores]
        mesh = Mesh(np.asarray(devices), ("core",))
        sm_kwargs = dict(mesh=mesh,
                         in_specs=(PartitionSpec("core"),) * (n_params + n_outs),
                         out_specs=(PartitionSpec("core"),) * n_outs)
        try:
            smapped = shard_map(_body, check_vma=False, **sm_kwargs)
        except TypeError:
            smapped = shard_map(_body, check_rep=False, **sm_kwargs)
        self.sharded = jax.jit(
            smapped,
            donate_argnums=tuple(range(n_params, n_params + n_outs)),
            keep_unused=True)
        self.sharding = NamedSharding(mesh, PartitionSpec("core"))

    def stage_inputs(self, in_maps):
        cat = [np.concatenate([np.asarray(in_maps[c][n])
                               for c in range(self.n_cores)], axis=0)
               for n in self.in_names]
        return [self.jax.device_put(a, self.sharding) for a in cat]

    def fresh_zeros(self):
        return [self.jax.device_put(
            np.zeros((self.n_cores * z.shape[0], *z.shape[1:]), z.dtype),
            self.sharding) for z in self.zero_outs]

    def run(self, in_maps):
        out_arrs = self.sharded(*self.stage_inputs(in_maps), *self.fresh_zeros())
        self.jax.block_until_ready(out_arrs)
        return self.unpack(out_arrs)

    def unpack(self, out_arrs):
        return [{n: np.asarray(out_arrs[i]).reshape(
                     self.n_cores, *self.out_avals[i].shape)[c]
                 for i, n in enumerate(self.out_names)}
                for c in range(self.n_cores)]


_CACHED = {}


def get_runner(n1_shard: int, n2: int, repeat: int = 1) -> Runner:
    key = (n1_shard, n2, repeat)
    if key not in _CACHED:
        _CACHED[key] = Runner(build_chamfer_bass(n1_shard, n2, repeat=repeat))
    return _CACHED[key]


def kernel(points1: np.ndarray, points2: np.ndarray) -> np.ndarray:
    p1 = np.asarray(points1)
    p2 = np.asarray(points2)
    runner = get_runner(p1.shape[0] // N_CORES, p2.shape[0])
    results = runner.run(make_aug_inputs(p1, p2))
    return combine_outputs(results)


def _numpy_ref(p1: np.ndarray, p2: np.ndarray, chunk: int = 1024) -> float:
    """Chunked numpy chamfer reference (f32 matmul expansion like the oracle)."""
    sq1 = (p1 * p1).sum(1)
    sq2 = (p2 * p2).sum(1)
    rowmin = np.full(p1.shape[0], np.inf, np.float32)
    colmin = np.full(p2.shape[0], np.inf, np.float32)
    for s in range(0, p1.shape[0], chunk):
        d2 = (sq1[s:s + chunk, None] + sq2[None, :]
              - 2.0 * (p1[s:s + chunk] @ p2.T))
        rowmin[s:s + chunk] = d2.min(1)
        np.minimum(colmin, d2.min(0), out=colmin)
    return float(np.sqrt(np.maximum(rowmin, 0)).sum()
                 + np.sqrt(np.maximum(colmin, 0)).sum())


if __name__ == "__main__":
    rng = np.random.default_rng(0)
    p1 = rng.standard_normal((N1_FULL, 3), dtype=np.float32)
    p2 = rng.standard_normal((N2_FULL, 3), dtype=np.float32)
    out = kernel(p1, p2)
    ref = _numpy_ref(p1, p2)
    print("kernel:", out, "ref:", ref, "rel:", abs(out - ref) / abs(ref))
